# revision 22
# baseline (speedup 1.0000x reference)
"""DenseNGCN layer on 8 Trainium2 NeuronCores.

Computes out = A @ (A @ (X W)) + b for a random sparse A (1.6M edges,
50k nodes), X [50k, 512], W [512, 64].

Strategy (1D node partitioning, bf16 pair-gather):
  - Nodes row-sharded across 8 cores (6250 rows/core, padded to 6272 =
    49 tiles of 128). Host permutes each core's local rows to balance
    per-tile edge-slot counts ("packing").
  - XW on TensorE per core (bf16), AllGather -> full projected table in
    DRAM as bf16. The table is addressed as 25088 PAIRS of rows (128
    bf16 = 256 B per pair), so a single int16 index region covers all
    50176 rows and the AllGather moves half the bytes of f32.
  - SPMM per iteration: per-edge PAIRS are fetched with dma_gather
    (256 B descriptors, 4096 indices per call), the needed half of each
    pair is selected+weighted on VectorE via two host-built masked vals
    arrays (w_e on the matching half, 0 on the other), then
    segment-summed on TensorE in two levels:
      L1: a constant "staircase" matrix sums groups of D=4 consecutive
          positions (slots) -- one matmul per 1024 edges.
      L2: per 128-slot K-tile, a one-hot matrix (built on VectorE from
          host-provided row ids) maps slot sums to the 128 rows of the
          dst tile; accumulated in PSUM.
  - AllGather the new table, repeat, add bias, write the shard out.

All per-core variation is data (indices/values/row-ids); the program is
identical across cores (SPMD).
"""

import dataclasses
import numpy as np

import concourse.bacc as bacc
import concourse.mybir as mybir
import concourse.tile as tile
from concourse.bass_utils import run_bass_kernel_spmd
from concourse.library_config import mlp as mlp_lib

F32 = mybir.dt.float32
BF16 = mybir.dt.bfloat16
I16 = mybir.dt.int16
BF16_NP = mybir.dt.np(BF16)

D = 4  # edges per slot


@dataclasses.dataclass
class Cfg:
    n_nodes: int = 50000
    n_edges: int = 1600000
    in_ch: int = 512
    out_ch: int = 64
    n_cores: int = 8
    n_tiles: int = 49       # dst tiles of 128 rows per core
    nktl: int = 424         # K-tiles (128 slots each) per core
    chunk: int = 4096       # edges per L1 bank
    gcall: int = 1024       # edges per dma_gather call
    n_queues: int = 4       # SWDGE queues (round-robin for gathers)
    dma_scratch: int = 16384
    iterations: int = 3

    @property
    def r_real(self):
        return self.n_nodes // self.n_cores

    @property
    def r_pad(self):
        return self.n_tiles * 128

    @property
    def w_ktl(self):
        return -(-self.nktl // self.n_tiles)

    @property
    def tile_caps(self):
        x = self.nktl - (self.w_ktl - 1) * self.n_tiles
        assert 0 < x <= self.n_tiles
        return np.array([self.w_ktl] * x + [self.w_ktl - 1] * (self.n_tiles - x))

    @property
    def ktl_base(self):
        return np.concatenate([[0], np.cumsum(self.tile_caps)])

    @property
    def tile_of_ktl(self):
        return np.repeat(np.arange(self.n_tiles), self.tile_caps)

    @property
    def nb(self):  # banks (8 ktiles each); nktl must divide evenly
        assert self.nktl % 8 == 0
        return self.nktl // 8

    @property
    def ep(self):  # edge positions total
        return self.nb * self.chunk

    @property
    def n_pairs(self):  # table row pairs (gather elements)
        return self.n_cores * self.r_pad // 2


CFG = Cfg()


# ------------------------------------------------------------------
# host preprocessing
# ------------------------------------------------------------------

def _balance_rows(slots, cfg):
    """Assign local rows to tiles; returns pos[] (row -> tile*128+pos).

    Greedy: rows sorted by slot count desc, placed in the feasible tile
    with most remaining slack. Caps: 128 rows, tile_caps[t]*128 slots.
    """
    nt = cfg.n_tiles
    caps = cfg.tile_caps * 128
    rows_left = np.full(nt, 128, dtype=np.int64)
    left = caps.astype(np.int64).copy()
    order = np.argsort(-slots, kind="stable")
    tile_of = np.full(cfg.r_real, -1, dtype=np.int64)
    for r in order:
        feas = (rows_left > 0) & (left >= slots[r])
        if not feas.any():
            raise RuntimeError("row packing failed; increase nktl")
        slack = np.where(feas, left, -1)
        t = int(np.argmax(slack))
        tile_of[r] = t
        rows_left[t] -= 1
        left[t] -= slots[r]
    pos = np.full(cfg.r_real, -1, dtype=np.int64)
    fill = np.zeros(nt, dtype=np.int64)
    for r in range(cfg.r_real):
        t = tile_of[r]
        pos[r] = t * 128 + fill[t]
        fill[t] += 1
    return pos


def preprocess(adj_index, adj_values, cfg=CFG):
    """Build per-core idx/valsA/valsB/rid arrays and row permutations."""
    rows = np.asarray(adj_index[0], dtype=np.int64)
    cols = np.asarray(adj_index[1], dtype=np.int64)
    vals = np.asarray(adj_values, dtype=np.float32)
    rr, rp, nt = cfg.r_real, cfg.r_pad, cfg.n_tiles

    core_of = rows // rr
    # pass 1: per-core slot counts and packing
    pos_all = []
    edge_data = []
    for c in range(cfg.n_cores):
        m = core_of == c
        rl = rows[m] - c * rr
        cl = cols[m]
        vl = vals[m]
        cnt = np.bincount(rl, minlength=rr)
        slots = -(-cnt // D)
        pos = _balance_rows(slots, cfg)
        pos_all.append(pos)
        edge_data.append((rl, cl, vl, slots))

    # pass 2: edge placement + gather indices
    # table row of source s: gpos = core_s*6272 + p_s*49 + t_s  (shards are
    # written partition-major: dram row p*nt + t), pair = gpos>>1.
    pos_cat = np.concatenate(pos_all)
    out = []
    for c in range(cfg.n_cores):
        rl, cl, vl, slots = edge_data[c]
        pos = pos_all[c]
        sc = cl // rr
        s_pos = pos_cat[sc * rr + (cl % rr)]
        t_s, p_s = s_pos // 128, s_pos % 128
        # table = [AllGather of tiles 0:25 | AllGather of tiles 25:49]
        gpos = np.where(
            t_s < 25, sc * 3200 + p_s * 25 + t_s,
            25600 + sc * 3072 + p_s * 24 + (t_s - 25))
        pair = gpos >> 1
        half = gpos & 1
        assert pair.max() < cfg.n_pairs <= 32768

        idx = np.zeros(cfg.ep, dtype=np.int16)
        vA = np.zeros(cfg.ep, dtype=np.float32)
        vB = np.zeros(cfg.ep, dtype=np.float32)
        rid = np.full((128, cfg.nktl), -1.0, dtype=np.float32)

        # slot base per packed position, per tile
        sl_of_pos = np.zeros(rp, dtype=np.int64)
        sl_of_pos[pos] = slots
        sl_pt = sl_of_pos.reshape(nt, 128)
        base_in_tile = np.cumsum(sl_pt, axis=1) - sl_pt  # [nt, 128]
        if ((base_in_tile[:, -1] + sl_pt[:, -1]) > cfg.tile_caps * 128).any():
            raise RuntimeError("tile slot overflow")

        pe = pos[rl]  # packed position of dst row
        te, pe_in = pe // 128, pe % 128
        # rank of edge within its dst row (stable order)
        o = np.argsort(pe, kind="stable")
        pe_s = pe[o]
        first = np.searchsorted(pe_s, pe_s)
        rank_s = np.arange(pe_s.size) - first
        rank = np.empty(pe.size, dtype=np.int64)
        rank[o] = rank_s

        si = base_in_tile[te, pe_in] + rank // D  # slot within tile
        w_in = si // 128
        q = si % 128
        a = cfg.ktl_base[te] + w_in               # global ktl
        bb, cc = a // 8, a % 8
        s_local = 1024 * bb + 256 * (q // 32) + 32 * cc + (q % 32)
        e = 4 * s_local + rank % D
        idx[e] = pair.astype(np.int16)
        vA[e] = np.where(half == 0, vl, 0.0)
        vB[e] = np.where(half == 1, vl, 0.0)

        # rid: slot -> packed row pos (within tile)
        for t in range(nt):
            ns = int(base_in_tile[t, -1] + sl_pt[t, -1])
            sia = np.arange(ns)
            owner = np.searchsorted(
                base_in_tile[t] + sl_pt[t], sia, side="right")
            aa = cfg.ktl_base[t] + sia // 128
            qq = sia % 128
            rid[qq, aa] = owner.astype(np.float32)

        out.append(dict(idx=np.tile(idx.reshape(-1, 16).T, (8, 1)).copy(),
                        valsA=vA.reshape(-1, 128).T.astype(BF16_NP).copy(),
                        valsB=vB.reshape(-1, 128).T.astype(BF16_NP).copy(),
                        rid=rid.astype(BF16_NP)))
    return out, pos_all


def stair_matrix():
    st = np.zeros((128, 32), dtype=np.float32)
    st[np.arange(128), np.arange(128) // D] = 1.0
    return st


# ------------------------------------------------------------------
# device program
# ------------------------------------------------------------------

def _bc_last(ap, n):
    return dataclasses.replace(ap, ap=list(ap.ap) + [[0, n]])


def build_program(cfg=CFG):
    nc = bacc.Bacc(None, target_bir_lowering=False, debug=False,
                   num_swdge_queues=cfg.n_queues,
                   dynamic_dma_scratch_size=cfg.dma_scratch)
    rp, nt = cfg.r_pad, cfg.n_tiles
    nb, ch = cfg.nb, cfg.chunk
    kc = cfg.in_ch // 128              # K chunks for XW
    ch_t = ch // 128                   # t-columns per gather chunk

    featT_d = nc.declare_dram_parameter("featT", [cfg.in_ch, rp], BF16, isOutput=False)
    w_d = nc.declare_dram_parameter("w", [cfg.in_ch, cfg.out_ch], BF16, isOutput=False)
    idx_d = nc.declare_dram_parameter("idx", [128, cfg.ep // 16], I16, isOutput=False)
    valsA_d = nc.declare_dram_parameter("valsA", [128, cfg.ep // 128], BF16, isOutput=False)
    valsB_d = nc.declare_dram_parameter("valsB", [128, cfg.ep // 128], BF16, isOutput=False)
    rid_d = nc.declare_dram_parameter("rid", [128, cfg.nktl], BF16, isOutput=False)
    stair_d = nc.declare_dram_parameter("stair", [128, 32], F32, isOutput=False)
    iota_d = nc.declare_dram_parameter("iota", [128, 128], BF16, isOutput=False)
    bias_d = nc.declare_dram_parameter("biasr", [128, cfg.out_ch], F32, isOutput=False)
    # output partition-major: dram row p*nt + t holds packed pos t*128+p
    out_d = nc.declare_dram_parameter("out", [rp, cfg.out_ch], F32, isOutput=True)

    shardA = [nc.dram_tensor(f"shardA{i}", [3200, cfg.out_ch], BF16) for i in range(2)]
    shardB = [nc.dram_tensor(f"shardB{i}", [3072, cfg.out_ch], BF16) for i in range(2)]
    table = [nc.dram_tensor(f"table{i}", [cfg.n_pairs, 2 * cfg.out_ch], BF16,
                            addr_space="Shared") for i in range(2)]
    groups = [list(range(cfg.n_cores))]

    def half_ag(i, stg_tile):
        """DMA tiles 0:25 of stg and AllGather into the first table half."""
        nc.sync.dma_start(
            shardA[i][:].rearrange("(p t) f -> p t f", p=128),
            stg_tile[:, 0:25, :])
        nc.gpsimd.collective_compute(
            "AllGather", mybir.AluOpType.bypass,
            ins=[shardA[i][:]], outs=[table[i][0:12800, :]],
            replica_groups=groups)

    def half_bg(i, stg_tile):
        """DMA tiles 25:49 of stg and AllGather into the second table half."""
        nc.sync.dma_start(
            shardB[i][:].rearrange("(p t) f -> p t f", p=128),
            stg_tile[:, 25:nt, :])
        nc.gpsimd.collective_compute(
            "AllGather", mybir.AluOpType.bypass,
            ins=[shardB[i][:]], outs=[table[i][12800:cfg.n_pairs, :]],
            replica_groups=groups)

    with tile.TileContext(nc) as tc:
        with tc.tile_pool(name="const", bufs=1) as constp:
            # dma_gather needs the mlp Q7 library resident
            nc.gpsimd.load_library(mlp_lib)
            stair_f = constp.tile([128, 32], F32)
            nc.sync.dma_start(stair_f[:], stair_d[:])
            stair = constp.tile([128, 32], BF16)
            nc.vector.tensor_copy(stair[:], stair_f[:])
            iota = constp.tile([128, 128], BF16)
            nc.sync.dma_start(iota[:], iota_d[:])
            rid = constp.tile([128, cfg.nktl], BF16)
            nc.sync.dma_start(rid[:], rid_d[:])
            valsA = constp.tile([128, cfg.ep // 128], BF16)
            nc.sync.dma_start(valsA[:], valsA_d[:])
            valsB = constp.tile([128, cfg.ep // 128], BF16)
            nc.sync.dma_start(valsB[:], valsB_d[:])
            idx = constp.tile([128, cfg.ep // 16], I16)
            nc.sync.dma_start(idx[:], idx_d[:])
            bias = constp.tile([128, cfg.out_ch], F32)
            nc.sync.dma_start(bias[:], bias_d[:])

            # ---------------- XW ----------------
            with (
                tc.tile_pool(name="feat", bufs=1) as featp,
                tc.tile_pool(name="xwps", bufs=2, space="PSUM") as xwps,
                tc.tile_pool(name="stg", bufs=1) as stgp,
            ):
                feat = featp.tile([128, kc, rp], BF16)
                nc.sync.dma_start(
                    feat[:], featT_d[:].rearrange("(a p) n -> p a n", p=128))
                wsb = featp.tile([128, kc, cfg.out_ch], BF16)
                nc.sync.dma_start(
                    wsb[:], w_d[:].rearrange("(a p) f -> p a f", p=128))
                stg1 = stgp.tile([128, nt, cfg.out_ch], BF16)
                for t in range(nt):
                    ps = xwps.tile([128, cfg.out_ch], F32, tag="xw", name=f"xw{t}")
                    for a in range(kc):
                        nc.tensor.matmul(
                            ps[:], feat[:, a, t * 128:(t + 1) * 128],
                            wsb[:, a, :], start=(a == 0), stop=(a == kc - 1))
                    nc.scalar.copy(stg1[:, t, :], ps[:])
                    if t == 24:
                        half_ag(0, stg1)
                half_bg(0, stg1)

            # ---------------- two SPMM iterations ----------------
            tile_of_ktl = cfg.tile_of_ktl
            tile_caps = cfg.tile_caps
            for it in range(cfg.iterations - 1):
                last = it == cfg.iterations - 2
                with (
                    tc.tile_pool(name=f"g{it}", bufs=6) as gpool,
                    tc.tile_pool(name=f"gsa{it}", bufs=2) as gsapool,
                    tc.tile_pool(name=f"gsb{it}", bufs=2) as gsbpool,
                    tc.tile_pool(name=f"srs{it}", bufs=4) as srspool,
                    tc.tile_pool(name=f"oh{it}", bufs=2) as ohpool,
                    tc.tile_pool(name=f"stg{it}", bufs=1) as stgp,
                    tc.tile_pool(name=f"l1ps{it}", bufs=3, space="PSUM") as l1ps,
                    tc.tile_pool(name=f"l2ps{it}", bufs=4, space="PSUM") as l2ps,
                ):
                    stg = stgp.tile([128, nt, cfg.out_ch], F32 if last else BF16,
                                    name=f"stg_{it}")
                    l2acc = {}
                    mm_done = [0] * nt
                    tbl = table[it]

                    qn = 0
                    for chk in range(nb):
                        g = gpool.tile([128, ch_t, 2 * cfg.out_ch], BF16, tag="g",
                                       name=f"g_{it}_{chk}")
                        gct = cfg.gcall // 128
                        for ci in range(ch // cfg.gcall):
                            i0 = (chk * ch + ci * cfg.gcall) // 16
                            nc.gpsimd.dma_gather(
                                g[:, ci * gct:(ci + 1) * gct, :], tbl[:],
                                idx[:, i0:i0 + cfg.gcall // 16],
                                cfg.gcall, cfg.gcall, 2 * cfg.out_ch,
                                single_packet=cfg.gcall <= 1024,
                                queue_num=qn % cfg.n_queues)
                            qn += 1
                        # half-select + weight: exactly one of vA/vB is
                        # nonzero per position
                        v0 = chk * ch_t
                        gsa = gsapool.tile([128, ch_t, cfg.out_ch], BF16,
                                           tag="gsa", name=f"gsa_{it}_{chk}")
                        nc.vector.tensor_tensor(
                            gsa[:], g[:, :, 0:cfg.out_ch],
                            _bc_last(valsA[:, v0:v0 + ch_t], cfg.out_ch),
                            mybir.AluOpType.mult)
                        gsb = gsbpool.tile([128, ch_t, cfg.out_ch], BF16,
                                           tag="gsb", name=f"gsb_{it}_{chk}")
                        nc.vector.tensor_tensor(
                            gsb[:], g[:, :, cfg.out_ch:2 * cfg.out_ch],
                            _bc_last(valsB[:, v0:v0 + ch_t], cfg.out_ch),
                            mybir.AluOpType.mult)

                        # L1: staircase sums D consecutive positions; the
                        # A/B halves are combined via PSUM accumulation
                        ps = l1ps.tile([128, 512], F32, tag="l1",
                                       name=f"l1_{it}_{chk}")
                        for j in range(4):
                            nc.tensor.matmul(
                                ps[32 * j:32 * j + 32, :], stair[:],
                                gsa[:, 8 * j:8 * j + 8, :]
                                .rearrange("p a f -> p (a f)"),
                                start=True, stop=False,
                                tile_position=(0, 32 * j))
                            nc.tensor.matmul(
                                ps[32 * j:32 * j + 32, :], stair[:],
                                gsb[:, 8 * j:8 * j + 8, :]
                                .rearrange("p a f -> p (a f)"),
                                start=False, stop=True,
                                tile_position=(0, 32 * j))
                        srs = srspool.tile([128, 512], BF16, tag="srs",
                                           name=f"srs_{it}_{chk}")
                        nc.scalar.copy(srs[:], ps[:])

                        # L2: one-hot maps slots to dst rows
                        oh = ohpool.tile([128, 8, 128], BF16, tag="oh",
                                         name=f"oh_{it}_{chk}")
                        kg0 = chk * 8
                        nc.vector.tensor_tensor(
                            oh[:], _bc_last(rid[:, kg0:kg0 + 8], 128),
                            dataclasses.replace(
                                iota[:], ap=[iota[:].ap[0], [0, 8], iota[:].ap[1]]),
                            mybir.AluOpType.is_equal)
                        for cc in range(8):
                            a = kg0 + cc
                            t = int(tile_of_ktl[a])
                            if t not in l2acc:
                                l2acc[t] = l2ps.tile(
                                    [128, cfg.out_ch], F32, tag="l2acc",
                                    name=f"l2acc_{it}_{t}")
                            nc.tensor.matmul(
                                l2acc[t][:], oh[:, cc, :],
                                srs[:, 64 * cc:64 * cc + 64],
                                start=(mm_done[t] == 0),
                                stop=(mm_done[t] == tile_caps[t] - 1))
                            mm_done[t] += 1
                            if mm_done[t] == tile_caps[t]:
                                if last:
                                    nc.vector.tensor_add(
                                        stg[:, t, :], l2acc[t][:], bias[:])
                                else:
                                    nc.vector.tensor_copy(stg[:, t, :], l2acc[t][:])
                                    if t == 24:
                                        half_ag(1, stg)
                                del l2acc[t]

                    if last:
                        nc.sync.dma_start(
                            out_d[:].rearrange("(p t) f -> p t f", p=128), stg[:])
                    else:
                        half_bg(1, stg)

    nc.compile()
    return nc


# ------------------------------------------------------------------
# host-side input/output marshalling
# ------------------------------------------------------------------

def make_in_maps(inputs, pre, pos_all, cfg=CFG):
    feats = np.asarray(inputs["features"], dtype=np.float32)
    wm = np.asarray(inputs["weight_matrix"], dtype=np.float32)
    bias = np.asarray(inputs["bias"], dtype=np.float32)
    st = stair_matrix()
    iota = np.tile(np.arange(128, dtype=np.float32), (128, 1)).astype(BF16_NP)
    bias_rep = np.tile(bias.reshape(1, cfg.out_ch), (128, 1)).astype(np.float32)
    w_bf = wm.astype(BF16_NP)
    in_maps = []
    for c in range(cfg.n_cores):
        fc = feats[c * cfg.r_real:(c + 1) * cfg.r_real]
        fp = np.zeros((cfg.r_pad, cfg.in_ch), dtype=np.float32)
        fp[pos_all[c]] = fc
        in_maps.append(dict(
            featT=np.ascontiguousarray(fp.T).astype(BF16_NP),
            w=w_bf, idx=pre[c]["idx"], valsA=pre[c]["valsA"],
            valsB=pre[c]["valsB"], rid=pre[c]["rid"],
            stair=st, iota=iota, biasr=bias_rep))
    return in_maps


_CACHE = {}


def kernel(adj_index, adj_values, features, weight_matrix, bias):
    cfg = CFG
    key = "prog"
    if key not in _CACHE:
        _CACHE[key] = build_program(cfg)
    nc = _CACHE[key]
    pre, pos_all = preprocess(adj_index, adj_values, cfg)
    in_maps = make_in_maps(
        dict(features=features, weight_matrix=weight_matrix, bias=bias),
        pre, pos_all, cfg)
    res = run_bass_kernel_spmd(nc, in_maps, core_ids=list(range(cfg.n_cores)))
    out = np.zeros((cfg.n_nodes, weight_matrix.shape[1]), dtype=np.float32)
    nt = cfg.n_tiles
    for c in range(cfg.n_cores):
        sh = res.results[c]["out"]  # row p*nt + t <-> packed pos t*128+p
        pos = pos_all[c]
        t, p = pos // 128, pos % 128
        out[c * cfg.r_real:(c + 1) * cfg.r_real] = sh[p * nt + t]
    return out
